# revision 35
# baseline (speedup 1.0000x reference)
"""Trainium2 Bass kernel for nn_CtxCrossConformerBlock (B=32,N=64,D=512,
H=4,Dh=128,J=2048,FF=2048,topk=64, local head pattern [1,4,8,*]).

Sharding: batch-parallel over 8 NeuronCores (4 batches/core), zero
collectives.

v2 layout (vs v1 baseline at 1245us/core cost-model):
- ctx FFN matmuls in fp8(e4m3, weights x16) with DoubleRow perf mode:
  halves PE time of the dominant GEMMs; x16 compensated in the Silu
  pre-scale (w1) and a PSUM-copy pre-scale (w2).
- dots/top-k/softmax numerics in bf16 (2x DVE throughput); exp folds
  the -thr into the ACT bias (pre-function semantics).
- LN stats via DVE bn_stats/bn_aggr (drops 2 ACT accumulation passes
  per LN tile).
- attention(b) is sliced and emitted interleaved with the ctx-FFN
  groups of batch b+1, so the DVE-heavy top-k hides under the PE-heavy
  FFN; vsb (V memory) is double-buffered, dots quadruple-buffered to
  make the overlap race-free; kT is consumed chunk-wise by the dots
  matmuls emitted inside the FFN phase.
- local band masks ship as per-core 0/1 bf16 tables (SPMD program is
  shared across cores) and are folded into the top-k keep mask on the
  otherwise-idle GpSimd/Pool engine.
"""
import contextlib

import numpy as np
import ml_dtypes

import concourse.bass as bass
import concourse.bacc as bacc
import concourse.mybir as mybir
from concourse import tile

BF = ml_dtypes.bfloat16
F8 = ml_dtypes.float8_e4m3fn
F32 = mybir.dt.float32
BF16 = mybir.dt.bfloat16
FP8 = mybir.dt.float8e4
AF = mybir.ActivationFunctionType
ALU = mybir.AluOpType
AX = mybir.AxisListType
DR = mybir.MatmulPerfMode.DoubleRow

B, N, DIM = 32, 64, 512
H, DH = 4, 128
J = B * N                      # 2048
FF = 2048
TOPK = 64
PATTERN = [1, 4, 8, None]
NCORES = 8
BLOC = B // NCORES             # 4 batches per core
P = 128
WSCALE = 16.0

_CACHE = {}


def build_bass():
    nc = bacc.Bacc()
    xin = nc.declare_dram_parameter("xin", [BLOC * N, DIM], F32,
                                    isOutput=False)
    ctxin = nc.declare_dram_parameter("ctxin", [BLOC * J, DIM], F32,
                                      isOutput=False)
    w1g_d = nc.declare_dram_parameter("w1g", [DIM, FF], BF16, isOutput=False)
    w2h_d = nc.declare_dram_parameter("w2h", [FF, DIM], BF16, isOutput=False)
    wc1_d = nc.declare_dram_parameter("wc1", [2 * P, 2 * FF], FP8,
                                      isOutput=False)
    wc2_d = nc.declare_dram_parameter("wc2", [8 * P, 2 * DIM], FP8,
                                      isOutput=False)
    wq_d = nc.declare_dram_parameter("wq", [DIM, DIM], BF16, isOutput=False)
    wk_d = nc.declare_dram_parameter("wk", [DIM, DIM], BF16, isOutput=False)
    wv_d = nc.declare_dram_parameter("wv", [DIM, DIM], BF16, isOutput=False)
    wo_d = nc.declare_dram_parameter("wo", [DIM, DIM], BF16, isOutput=False)
    id_d = nc.declare_dram_parameter("ident", [P, P], BF16, isOutput=False)
    id8_d = nc.declare_dram_parameter("ident8", [P, P], FP8, isOutput=False)
    band_d = nc.declare_dram_parameter("bands", [BLOC * 2 * P, J], BF16,
                                       isOutput=False)
    outd = nc.declare_dram_parameter("out", [BLOC * N, DIM], F32,
                                     isOutput=True)

    with tile.TileContext(nc) as tc, contextlib.ExitStack() as ctx:
        wp = ctx.enter_context(tc.tile_pool(name="w", bufs=1))
        pers = ctx.enter_context(tc.tile_pool(name="pers", bufs=1))
        ctp = ctx.enter_context(tc.tile_pool(name="ct", bufs=8))
        lnp = ctx.enter_context(tc.tile_pool(name="ln", bufs=2))
        ttp = ctx.enter_context(tc.tile_pool(name="tt", bufs=2))
        statp = ctx.enter_context(tc.tile_pool(name="stat", bufs=3))
        dotp = ctx.enter_context(tc.tile_pool(name="dot", bufs=4))
        scrp = ctx.enter_context(tc.tile_pool(name="scr", bufs=2))
        smp = ctx.enter_context(tc.tile_pool(name="sm", bufs=2))
        aop = ctx.enter_context(tc.tile_pool(name="ao", bufs=2))
        psT = ctx.enter_context(tc.tile_pool(name="psT", bufs=2,
                                             space="PSUM"))
        psH = ctx.enter_context(tc.tile_pool(name="psH", bufs=2,
                                             space="PSUM"))
        psAV = ctx.enter_context(tc.tile_pool(name="psAV", bufs=2,
                                              space="PSUM"))
        psD = ctx.enter_context(tc.tile_pool(name="psD", bufs=2,
                                             space="PSUM"))

        def _ln_stats(src, rows, ncols):
            """mean/var via DVE bn_stats; returns (inv, nmi) [P,1] f32."""
            r = rows
            st6 = statp.tile([P, 6], F32, tag="st6", name="st6")
            mv = statp.tile([P, 2], F32, tag="mv", name="mv")
            inv = statp.tile([P, 1], F32, tag="inv", name="inv")
            nmi = statp.tile([P, 1], F32, tag="nmi", name="nmi")
            nc.vector.bn_stats(out=st6[r, :], in_=src[r, 0:ncols])
            nc.vector.bn_aggr(out=mv[r, :], in_=st6[r, :])
            # inv = 1/sqrt(var+eps)
            nc.vector.tensor_scalar_add(inv[r, :], mv[r, 1:2], 1e-5)
            nc.scalar.activation(inv[r, :], inv[r, :], AF.Sqrt)
            nc.vector.reciprocal(inv[r, :], inv[r, :])
            # nmi = -mean*inv
            nc.vector.tensor_tensor(nmi[r, :], mv[r, 0:1], inv[r, :],
                                    op=ALU.mult)
            nc.vector.tensor_scalar_mul(nmi[r, :], nmi[r, :], -1.0)
            return inv, nmi

        def _ln_apply(dst, src, rows, inv, nmi):
            nc.scalar.activation(dst[rows, :], src[rows, :], AF.Identity,
                                 scale=inv[rows, :], bias=nmi[rows, :])

        def transpose4(src, dst_t, dst_col, kstride, ncols=P, ident_n=P):
            """token-major [128,512] -> 4 feature-major chunks into dst_t,
            chunk kc at cols [kc*kstride + dst_col ...)."""
            pt = psT.tile([P, 512], src.dtype, tag="pst", name="pst")
            idt = ident8 if src.dtype == FP8 else ident
            for kc in range(4):
                nc.tensor.transpose(pt[:, kc * P:kc * P + ncols],
                                    src[:, kc * P:(kc + 1) * P],
                                    idt[0:ident_n, 0:ident_n])
            src3 = pt.rearrange("p (k c) -> p k c", c=P)[:, :, 0:ncols]
            dst3 = dst_t.rearrange("p (k c) -> p k c",
                                   c=kstride)[:, :, dst_col:dst_col + ncols]
            nc.scalar.activation(dst3, src3, AF.Copy)

        # ---- weights ----
        wc1t = [wp.tile([P, 2 * FF], FP8, tag=f"wc1{i}", name=f"wc1{i}")
                for i in range(2)]
        wc2t = [wp.tile([P, 2 * DIM], FP8, tag=f"wc2{i}", name=f"wc2{i}")
                for i in range(8)]
        wq = [wp.tile([P, DIM], BF16, tag=f"wq{i}", name=f"wq{i}")
              for i in range(4)]
        wk = [wp.tile([P, DIM], BF16, tag=f"wk{i}", name=f"wk{i}")
              for i in range(4)]
        wv = [wp.tile([P, DIM], BF16, tag=f"wv{i}", name=f"wv{i}")
              for i in range(4)]
        wo = [wp.tile([P, DIM], BF16, tag=f"wo{i}", name=f"wo{i}")
              for i in range(4)]
        ident = wp.tile([P, P], BF16, tag="ident", name="ident")
        nc.sync.dma_start(ident[:, :], id_d[:, :])
        sc16 = wp.tile([P, 1], F32, tag="sc16", name="sc16")
        nc.vector.memset(sc16[:, :], 1.0 / WSCALE)
        ident8 = wp.tile([P, P], FP8, tag="ident8", name="ident8")
        nc.sync.dma_start(ident8[:, :], id8_d[:, :])
        # x-FFN weights live in the attention-scratch rings (those rings
        # first rotate into real use only after the x-FFN has consumed
        # these tiles).
        w1g = [scrp.tile([P, J], BF16, tag="w", name=f"w1g{i}", bufs=2)
               for i in range(2)] + \
              [scrp.tile([P, J], BF16, tag="em", name=f"w1g{i+2}", bufs=2)
               for i in range(2)]
        w2p = [scrp.tile([P, J], BF16, tag="ge", name=f"w2p{i}", bufs=2)
               for i in range(2)] + \
              [scrp.tile([P, J], BF16, tag="attnf", name=f"w2p{i+2}",
                         bufs=2) for i in range(2)]
        w2h = [w2p[m // 4][:, (m % 4) * DIM:(m % 4 + 1) * DIM]
               for m in range(16)]
        # ctx-FFN weights first (group 0 starts on them), then the rest
        for i in range(2):
            nc.sync.dma_start(wc1t[i][:, :], wc1_d[i * P:(i + 1) * P, :])
        for i in range(8):
            nc.sync.dma_start(wc2t[i][:, :], wc2_d[i * P:(i + 1) * P, :])
        for i in range(4):
            nc.sync.dma_start(wk[i][:, :], wk_d[i * P:(i + 1) * P, :])
            nc.sync.dma_start(wv[i][:, :], wv_d[i * P:(i + 1) * P, :])
        for i in range(4):
            nc.sync.dma_start(w1g[i][:, 0:FF], w1g_d[i * P:(i + 1) * P, :])
            nc.sync.dma_start(wq[i][:, :], wq_d[i * P:(i + 1) * P, :])
        for m in range(16):
            nc.sync.dma_start(w2h[m], w2h_d[m * P:(m + 1) * P, :])
        for i in range(4):
            nc.sync.dma_start(wo[i][:, :], wo_d[i * P:(i + 1) * P, :])

        # persistent activations
        x2 = [pers.tile([P, DIM], F32, tag=f"x2_{t}", name=f"x2_{t}")
              for t in range(2)]
        qT = [pers.tile([P, BLOC * N], BF16, tag=f"qT{h}", name=f"qT{h}")
              for h in range(H)]
        kT = [pers.tile([P, J], BF16, tag=f"kT{h}", name=f"kT{h}")
              for h in range(H)]
        vsb = [pers.tile([P, 16 * DIM], BF16, tag=f"vsb{t}", name=f"vsb{t}")
               for t in range(2)]
        swT = [pers.tile([P, 2 * 512], FP8, tag=f"swT{m}", name=f"swT{m}")
               for m in range(8)]

        def emit_xffn_q():
            xt = [ctp.tile([P, DIM], F32, tag="ct", name="ct")
                  for _ in range(2)]
            for t in range(2):
                nc.sync.dma_start(xt[t][:, :], xin[t * P:(t + 1) * P, :])
            lnTx = ttp.tile([P, 4 * 256], BF16, tag="lnTx", name="lnTx",
                            bufs=1)
            for t in range(2):
                lno = lnp.tile([P, DIM], BF16, tag="lnout", name="lnout")
                inv, nmi = _ln_stats(xt[t], slice(0, P), DIM)
                _ln_apply(lno, xt[t], slice(0, P), inv, nmi)
                transpose4(lno, lnTx, t * P, 256)
            swx = [lnp.tile([P, 256], BF16, tag=f"swx{m}", name=f"swx{m}",
                            bufs=1) for m in range(16)]
            for m in range(16):
                hps = psH.tile([P, 512], F32, tag="psh", name="psh")
                for kc in range(4):
                    nc.tensor.matmul(
                        hps[:, 0:256],
                        w1g[kc][:, m * P:(m + 1) * P],
                        lnTx[:, kc * 256:(kc + 1) * 256],
                        start=(kc == 0), stop=(kc == 3))
                nc.scalar.activation(swx[m][:, :], hps[:, 0:256], AF.Silu)
            for t in range(2):
                fps = psH.tile([P, 512], F32, tag="psh", name="psh")
                for m in range(16):
                    nc.tensor.matmul(fps[:, :],
                                     swx[m][:, t * P:(t + 1) * P],
                                     w2h[m],
                                     start=(m == 0), stop=(m == 15))
                nc.vector.tensor_tensor(x2[t][:, :], fps[:, :],
                                        xt[t][:, :], op=ALU.add)
            aT = ttp.tile([P, 4 * 256], BF16, tag="lnTx", name="aT",
                          bufs=1)
            for t in range(2):
                a_bf = lnp.tile([P, DIM], BF16, tag="lnout", name="lnout")
                inv, nmi = _ln_stats(x2[t], slice(0, P), DIM)
                _ln_apply(a_bf, x2[t], slice(0, P), inv, nmi)
                transpose4(a_bf, aT, t * P, 256)
            for h in range(H):
                qps = psH.tile([P, 512], F32, tag="psh", name="psh")
                for kc in range(4):
                    nc.tensor.matmul(qps[:, 0:256],
                                     wq[kc][:, h * P:(h + 1) * P],
                                     aT[:, kc * 256:(kc + 1) * 256],
                                     start=(kc == 0), stop=(kc == 3))
                nc.scalar.activation(qT[h][:, :], qps[:, 0:256], AF.Copy)

        dots_t = {}   # (b, pair) -> tile

        def emit_group(b, g, hook=None):
            """ctx FFN + KV projection + dots chunk for batch b, group g
            (512 ctx tokens). fp8 DoubleRow for the FFN GEMMs. `hook` is
            called mid-group (after the Silu stage) to interleave
            attention work into the engine queues at fine granularity."""
            base = b * J + g * 512
            cts = [ctp.tile([P, DIM], F32, tag="ct", name="ct")
                   for _ in range(4)]
            for r in range(4):
                nc.sync.dma_start(
                    cts[r][:, :],
                    ctxin[base + r * P: base + (r + 1) * P, :])
            lnT = ttp.tile([P, 4 * 512], FP8, tag="lnTc", name="lnTc")
            for r in range(4):
                lno = lnp.tile([P, DIM], BF16, tag="lnout", name="lnout")
                inv, nmi = _ln_stats(cts[r], slice(0, P), DIM)
                _ln_apply(lno, cts[r], slice(0, P), inv, nmi)
                transpose4(lno, lnT, r * P, 512)
            for m in range(16):
                hps = psH.tile([P, 512], F32, tag="psh", name="psh")
                for k2 in range(2):
                    lhs = wc1t[k2].rearrange("p (two m) -> p two m",
                                             two=2)[:, :, m * P:(m + 1) * P]
                    rhs = lnT[:, k2 * 1024:(k2 + 1) * 1024].rearrange(
                        "p (two n) -> p two n", two=2)
                    nc.tensor.matmul(hps[:, :], lhs, rhs,
                                     start=(k2 == 0), stop=(k2 == 1),
                                     perf_mode=DR)
                nc.scalar.activation(swT[m // 2][:, (m % 2) * 512:
                                                 (m % 2 + 1) * 512],
                                     hps[:, :], AF.Silu,
                                     scale=sc16[:, :])
            if hook is not None:
                hook()
            c2s = [lnp.tile([P, DIM], BF16, tag="c2o", name="c2o")
                   for _ in range(4)]
            for t in range(4):
                fps = psH.tile([P, 512], F32, tag="psh", name="psh")
                for mp in range(8):
                    lhs = swT[mp].rearrange(
                        "p (two n) -> p two n",
                        two=2)[:, :, t * P:(t + 1) * P]
                    rhs = wc2t[mp].rearrange("p (two n) -> p two n", two=2)
                    nc.tensor.matmul(fps[:, :], lhs, rhs,
                                     start=(mp == 0), stop=(mp == 7),
                                     perf_mode=DR)
                tmp = lnp.tile([P, DIM], F32, tag="ftmp", name="ftmp")
                nc.scalar.activation(tmp[:, :], fps[:, :], AF.Copy,
                                     scale=sc16[:, :])
                nc.vector.tensor_tensor(c2s[t][:, :], tmp[:, :],
                                        cts[t][:, :], op=ALU.add)
            c2T = ttp.tile([P, 4 * 512], BF16, tag="c2T", name="c2T")
            for r in range(4):
                transpose4(c2s[r], c2T, r * P, 512)
            for h in range(H):
                kps = psH.tile([P, 512], F32, tag="psh", name="psh")
                for kc in range(4):
                    nc.tensor.matmul(kps[:, :],
                                     wk[kc][:, h * P:(h + 1) * P],
                                     c2T[:, kc * 512:(kc + 1) * 512],
                                     start=(kc == 0), stop=(kc == 3))
                nc.scalar.activation(kT[h][:, g * 512:(g + 1) * 512],
                                     kps[:, :], AF.Copy)
            vdst = vsb[b % 2]
            for t in range(4):
                vps = psH.tile([P, 512], F32, tag="psh", name="psh")
                for kc in range(4):
                    nc.tensor.matmul(
                        vps[:, :],
                        c2T[:, kc * 512 + t * P:kc * 512 + (t + 1) * P],
                        wv[kc][:, :],
                        start=(kc == 0), stop=(kc == 3))
                rt = g * 4 + t
                nc.vector.tensor_copy(vdst[:, rt * DIM:(rt + 1) * DIM],
                                      vps[:, :])
            if b > 0:
                emit_dots(b, g)

        def emit_dots(b, g):
            for pair in range(2):
                if g == 0:
                    if pair == 0:
                        dots_t[(b, 0)] = dotp.tile([P, J], BF16,
                                                   tag="dots", name="dots")
                    else:
                        dots_t[(b, 1)] = dotp.tile([P, J], BF16,
                                                   tag="dots", name="dots")
                dots = dots_t[(b, pair)]
                for hi in range(2):
                    h = 2 * pair + hi
                    dps = psD.tile([64, 512], F32, tag="psd", name="psd")
                    nc.tensor.matmul(dps[:, :],
                                     qT[h][:, b * N:(b + 1) * N],
                                     kT[h][:, g * 512:(g + 1) * 512],
                                     start=True, stop=True)
                    nc.scalar.activation(
                        dots[hi * 64:(hi + 1) * 64,
                             g * 512:(g + 1) * 512],
                        dps[:, :], AF.Copy)

        att_st = {}   # (b, pair) -> dict of tiles across slices

        def emit_topk_part(b, pair, part):
            """part 0: copy + rounds 0-3; part 1: rounds 4-7 + thr + ge."""
            dots = dots_t[(b, pair)]
            if part == 0:
                st = att_st[(b, pair)] = {}
                w = scrp.tile([P, J], BF16, tag="w", name="w")
                nc.gpsimd.tensor_copy(w[:, :], dots[:, :])
                mx = smp.tile([P, 64], BF16, tag="mx", name="mx")
                st['w'] = w
                st['mx'] = mx
                rounds = range(0, 4)
            else:
                st = att_st[(b, pair)]
                w, mx = st['w'], st['mx']
                rounds = range(4, 8)
            for r8 in rounds:
                nc.vector.max(mx[:, r8 * 8:(r8 + 1) * 8], w[:, :])
                if r8 < 7:
                    nc.vector.match_replace(
                        w[:, :], mx[:, r8 * 8:(r8 + 1) * 8], w[:, :],
                        -3.0e38)
            if part == 0:
                # prefetch band mask while rounds run
                band = scrp.tile([P, J], BF16, tag="ge", name="band")
                nc.sync.dma_start(
                    band[:, :],
                    band_d[(b * 2 + pair) * P:(b * 2 + pair + 1) * P, :])
                st['band'] = band
                return
            thr = mx[:, 63:64]
            thrf = smp.tile([P, 1], F32, tag="thrf", name="thrf")
            nc.vector.tensor_scalar_mul(thrf[:, :], thr, 1.0)
            negthr = smp.tile([P, 1], F32, tag="negthr", name="negthr")
            nc.vector.tensor_scalar_mul(negthr[:, :], thr, -1.0)
            st['negthr'] = negthr
            ge = scrp.tile([P, J], BF16, tag="attnf", name="ge")
            nc.vector.tensor_scalar(ge[:, :], dots[:, :], thrf[:, :], None,
                                    op0=ALU.is_ge)
            nc.gpsimd.tensor_tensor(ge[:, :], ge[:, :], st['band'][:, :],
                                    op=ALU.mult)
            st['ge'] = ge

        def emit_soft(b, pair, aout):
            dots = dots_t[(b, pair)]
            st = att_st[(b, pair)]
            em = scrp.tile([P, J], BF16, tag="em", name="em")
            nc.scalar.activation(em[:, :], dots[:, :], AF.Exp,
                                 bias=st['negthr'][:, :])
            nc.vector.tensor_tensor(em[:, :], em[:, :], st['ge'][:, :],
                                    op=ALU.mult)
            zS = smp.tile([P, 1], F32, tag="z", name="z")
            nc.vector.reduce_sum(zS[:, :], em[:, :], axis=AX.X)
            degS = smp.tile([P, 1], F32, tag="deg", name="deg")
            izS = smp.tile([P, 1], F32, tag="iz", name="iz")
            uS = smp.tile([P, 1], F32, tag="u", name="u")
            nc.vector.tensor_scalar(degS[:, :], zS[:, :], 0.5, None,
                                    op0=ALU.is_le)
            nc.vector.tensor_tensor(izS[:, :], zS[:, :], degS[:, :],
                                    op=ALU.add)
            nc.vector.reciprocal(izS[:, :], izS[:, :])
            nc.vector.tensor_scalar_mul(uS[:, :], degS[:, :], 1.0 / J)
            attnf = scrp.tile([P, J], BF16, tag="w", name="attnf")
            nc.vector.tensor_scalar(attnf[:, :], em[:, :], izS[:, :],
                                    uS[:, :], op0=ALU.mult, op1=ALU.add)
            # transpose attnf -> atT
            atT = scrp.tile([P, J], BF16, tag="em", name="atT")
            for jcg in range(4):
                pt = psT.tile([P, 512], BF16, tag="pst", name="pst")
                for j4 in range(4):
                    jc = jcg * 4 + j4
                    nc.tensor.transpose(pt[:, j4 * P:(j4 + 1) * P],
                                        attnf[:, jc * P:(jc + 1) * P],
                                        ident[:, :])
                nc.scalar.activation(atT[:, jcg * 512:(jcg + 1) * 512],
                                     pt[:, :], AF.Copy)
            avp = [psAV.tile([64, 512], F32, tag="av", name="av")
                   for _ in range(2)]
            vsrc = vsb[b % 2]
            for jc in range(16):
                for hi in range(2):
                    h = 2 * pair + hi
                    nc.tensor.matmul(
                        avp[hi][:, 0:P],
                        atT[:, jc * P + hi * 64:jc * P + hi * 64 + 64],
                        vsrc[:, jc * DIM + h * P:jc * DIM + (h + 1) * P],
                        start=(jc == 0), stop=(jc == 15))
            for hi in range(2):
                h = 2 * pair + hi
                nc.scalar.activation(aout[:, h * P:(h + 1) * P],
                                     avp[hi][:, 0:P], AF.Copy)

        def emit_out(b, aout):
            aoT = aop.tile([P, 256], BF16, tag="aoT", name="aoT")
            pt = psT.tile([P, 512], BF16, tag="pst", name="pst")
            for kc in range(4):
                nc.tensor.transpose(pt[:, kc * P:kc * P + 64],
                                    aout[:, kc * P:(kc + 1) * P],
                                    ident[0:64, 0:64])
            src3 = pt.rearrange("p (k c) -> p k c", c=P)[:, :, 0:64]
            dst3 = aoT.rearrange("p (k c) -> p k c", c=64)
            nc.scalar.activation(dst3, src3, AF.Copy)
            ops = psAV.tile([64, 512], F32, tag="av", name="av")
            for kc in range(4):
                nc.tensor.matmul(ops[:, :], aoT[:, kc * 64:(kc + 1) * 64],
                                 wo[kc][:, :],
                                 start=(kc == 0), stop=(kc == 3))
            xf = aop.tile([64, DIM], F32, tag="xf", name="xf")
            x2t = x2[b // 2]
            nc.vector.tensor_tensor(
                xf[:, :], ops[:, :],
                x2t[(b % 2) * 64:(b % 2) * 64 + 64, :], op=ALU.add)
            outn = aop.tile([64, DIM], F32, tag="outn", name="outn")
            inv, nmi = _ln_stats(xf, slice(0, 64), DIM)
            _ln_apply(outn, xf, slice(0, 64), inv, nmi)
            nc.sync.dma_start(outd[b * N:(b + 1) * N, :], outn[0:64, :])

        def att_slice(b, s, aout_t):
            """8 half-group-granularity slices per batch."""
            if s == 0:
                emit_topk_part(b, 0, 0)
            elif s == 1:
                emit_topk_part(b, 0, 1)
            elif s == 2:
                emit_soft(b, 0, aout_t[b])
            elif s == 3:
                emit_topk_part(b, 1, 0)
            elif s == 4:
                emit_topk_part(b, 1, 1)
            elif s == 5:
                emit_soft(b, 1, aout_t[b])
            elif s == 6:
                emit_out(b, aout_t[b])

        aout_t = {}
        for b in range(BLOC):
            aout_t[b] = aop.tile([64, 512], BF16, tag="aout", name="aout")
            for g in range(4):
                if b > 0:
                    emit_group(b, g, hook=(
                        lambda s=2 * g: att_slice(b - 1, s, aout_t)))
                    att_slice(b - 1, 2 * g + 1, aout_t)
                else:
                    emit_group(b, g)
            if b == 0:
                emit_xffn_q()
                for g in range(4):
                    emit_dots(0, g)
        for s in range(8):
            att_slice(BLOC - 1, s, aout_t)
    nc.compile()
    return nc


def _fold_weights(inputs):
    f32 = np.float32
    g1 = np.asarray(inputs['ln1_g'], f32)[:, None]
    gkv = np.asarray(inputs['lnkv_g'], f32)[:, None]
    ga = np.asarray(inputs['lna_g'], f32)[:, None]
    w1g = (g1 * np.asarray(inputs['ff1_w1'], f32)).astype(BF)
    w2h = (0.5 * np.asarray(inputs['ff1_w2'], f32)).astype(BF)
    wc1 = (gkv * np.asarray(inputs['ffkv_w1'], f32) * WSCALE)
    wc1 = wc1.reshape(2, 2, P, FF).transpose(0, 2, 1, 3).reshape(
        2 * P, 2 * FF).astype(F8)
    wc2 = (0.5 * np.asarray(inputs['ffkv_w2'], f32) * WSCALE)
    wc2 = wc2.reshape(8, 2, P, DIM).transpose(0, 2, 1, 3).reshape(
        8 * P, 2 * DIM).astype(F8)
    wq = (ga * np.asarray(inputs['wq'], f32) * (DH ** -0.5)).astype(BF)
    wkv = np.asarray(inputs['wkv'], f32)
    wk = np.ascontiguousarray(wkv[:, :DIM]).astype(BF)
    wv = np.ascontiguousarray(wkv[:, DIM:]).astype(BF)
    wo = np.asarray(inputs['wo'], f32).astype(BF)
    return w1g, w2h, wc1, wc2, wq, wk, wv, wo


def _band_table(core):
    """Multiplicative 0/1 local-band masks for this core's 4 batches x 2
    head-pairs: [BLOC*2*128, J] bf16, rows (b*2+pair)*128 + hi*64 + i."""
    m = np.ones((BLOC * 2 * P, J), np.float32)
    blk = np.arange(J) // N
    for b in range(BLOC):
        gbat = core * BLOC + b
        for pair in range(2):
            for hi in range(2):
                h = 2 * pair + hi
                L = PATTERN[h]
                if L is None:
                    continue
                bad = np.abs(blk - gbat) > L
                r0 = (b * 2 + pair) * P + hi * 64
                m[r0:r0 + 64, bad] = 0.0
    return m.astype(BF)


def _in_maps(inputs):
    x = np.asarray(inputs['x'], np.float32)
    ctxf = np.asarray(inputs['context'], np.float32)
    w1g, w2h, wc1, wc2, wq, wk, wv, wo = _fold_weights(inputs)
    ident = np.eye(P, dtype=BF)
    ident8 = np.eye(P, dtype=F8)
    in_maps = []
    for c in range(NCORES):
        bs = slice(c * BLOC, (c + 1) * BLOC)
        in_maps.append({
            'xin': np.ascontiguousarray(x[bs].reshape(BLOC * N, DIM)),
            'ctxin': np.ascontiguousarray(ctxf[bs].reshape(BLOC * J, DIM)),
            'w1g': w1g, 'w2h': w2h, 'wc1': wc1, 'wc2': wc2,
            'wq': wq, 'wk': wk, 'wv': wv, 'wo': wo,
            'ident': ident, 'ident8': ident8,
            'bands': _band_table(c),
        })
    return in_maps


def kernel(**inputs):
    from concourse.bass_utils import run_bass_kernel_spmd

    if 'nc' not in _CACHE:
        _CACHE['nc'] = build_bass()
    nc = _CACHE['nc']
    res = run_bass_kernel_spmd(nc, _in_maps(inputs), list(range(NCORES)))
    outs = [np.asarray(res.results[c]['out']).reshape(BLOC, N, DIM)
            for c in range(NCORES)]
    on = np.concatenate(outs, axis=0)
    g = np.asarray(inputs['lnf_g'], np.float32)
    bta = np.asarray(inputs['lnf_b'], np.float32)
    return (g * on + bta).astype(np.float32)


# revision 36
# speedup vs baseline: 1.0218x; 1.0218x over previous
"""Trainium2 Bass kernel for nn_CtxCrossConformerBlock (B=32,N=64,D=512,
H=4,Dh=128,J=2048,FF=2048,topk=64, local head pattern [1,4,8,*]).

Sharding: batch-parallel over 8 NeuronCores (4 batches/core), zero
collectives.

v2 layout (vs v1 baseline at 1245us/core cost-model):
- ctx FFN matmuls in fp8(e4m3, weights x16) with DoubleRow perf mode:
  halves PE time of the dominant GEMMs; x16 compensated in the Silu
  pre-scale (w1) and a PSUM-copy pre-scale (w2).
- dots/top-k/softmax numerics in bf16 (2x DVE throughput); exp folds
  the -thr into the ACT bias (pre-function semantics).
- LN stats via DVE bn_stats/bn_aggr (drops 2 ACT accumulation passes
  per LN tile).
- attention(b) is sliced and emitted interleaved with the ctx-FFN
  groups of batch b+1, so the DVE-heavy top-k hides under the PE-heavy
  FFN; vsb (V memory) is double-buffered, dots quadruple-buffered to
  make the overlap race-free; kT is consumed chunk-wise by the dots
  matmuls emitted inside the FFN phase.
- local band masks ship as per-core 0/1 bf16 tables (SPMD program is
  shared across cores) and are folded into the top-k keep mask on the
  otherwise-idle GpSimd/Pool engine.
"""
import contextlib

import numpy as np
import ml_dtypes

import concourse.bass as bass
import concourse.bacc as bacc
import concourse.mybir as mybir
from concourse import tile

BF = ml_dtypes.bfloat16
F8 = ml_dtypes.float8_e4m3fn
F32 = mybir.dt.float32
BF16 = mybir.dt.bfloat16
FP8 = mybir.dt.float8e4
AF = mybir.ActivationFunctionType
ALU = mybir.AluOpType
AX = mybir.AxisListType
DR = mybir.MatmulPerfMode.DoubleRow

B, N, DIM = 32, 64, 512
H, DH = 4, 128
J = B * N                      # 2048
FF = 2048
TOPK = 64
PATTERN = [1, 4, 8, None]
NCORES = 8
BLOC = B // NCORES             # 4 batches per core
P = 128
WSCALE = 16.0

_CACHE = {}


def build_bass():
    nc = bacc.Bacc()
    xin = nc.declare_dram_parameter("xin", [BLOC * N, DIM], F32,
                                    isOutput=False)
    ctxin = nc.declare_dram_parameter("ctxin", [BLOC * J, DIM], F32,
                                      isOutput=False)
    w1g_d = nc.declare_dram_parameter("w1g", [DIM, FF], BF16, isOutput=False)
    w2h_d = nc.declare_dram_parameter("w2h", [FF, DIM], BF16, isOutput=False)
    wc1_d = nc.declare_dram_parameter("wc1", [2 * P, 2 * FF], FP8,
                                      isOutput=False)
    wc2_d = nc.declare_dram_parameter("wc2", [8 * P, 2 * DIM], FP8,
                                      isOutput=False)
    wq_d = nc.declare_dram_parameter("wq", [DIM, DIM], BF16, isOutput=False)
    wk_d = nc.declare_dram_parameter("wk", [DIM, DIM], BF16, isOutput=False)
    wv_d = nc.declare_dram_parameter("wv", [DIM, DIM], BF16, isOutput=False)
    wo_d = nc.declare_dram_parameter("wo", [DIM, DIM], BF16, isOutput=False)
    id_d = nc.declare_dram_parameter("ident", [P, P], BF16, isOutput=False)
    id8_d = nc.declare_dram_parameter("ident8", [P, P], FP8, isOutput=False)
    band_d = nc.declare_dram_parameter("bands", [BLOC * 2 * P, J], BF16,
                                       isOutput=False)
    outd = nc.declare_dram_parameter("out", [BLOC * N, DIM], F32,
                                     isOutput=True)

    with tile.TileContext(nc) as tc, contextlib.ExitStack() as ctx:
        wp = ctx.enter_context(tc.tile_pool(name="w", bufs=1))
        pers = ctx.enter_context(tc.tile_pool(name="pers", bufs=1))
        ctp = ctx.enter_context(tc.tile_pool(name="ct", bufs=8))
        lnp = ctx.enter_context(tc.tile_pool(name="ln", bufs=2))
        ttp = ctx.enter_context(tc.tile_pool(name="tt", bufs=2))
        statp = ctx.enter_context(tc.tile_pool(name="stat", bufs=3))
        dotp = ctx.enter_context(tc.tile_pool(name="dot", bufs=4))
        scrp = ctx.enter_context(tc.tile_pool(name="scr", bufs=2))
        smp = ctx.enter_context(tc.tile_pool(name="sm", bufs=2))
        aop = ctx.enter_context(tc.tile_pool(name="ao", bufs=2))
        psT = ctx.enter_context(tc.tile_pool(name="psT", bufs=2,
                                             space="PSUM"))
        psH = ctx.enter_context(tc.tile_pool(name="psH", bufs=2,
                                             space="PSUM"))
        psAV = ctx.enter_context(tc.tile_pool(name="psAV", bufs=2,
                                              space="PSUM"))
        psD = ctx.enter_context(tc.tile_pool(name="psD", bufs=2,
                                             space="PSUM"))

        def _ln_stats(src, rows, ncols):
            """mean/var via DVE bn_stats; returns (inv, nmi) [P,1] f32."""
            r = rows
            st6 = statp.tile([P, 6], F32, tag="st6", name="st6")
            mv = statp.tile([P, 2], F32, tag="mv", name="mv")
            inv = statp.tile([P, 1], F32, tag="inv", name="inv")
            nmi = statp.tile([P, 1], F32, tag="nmi", name="nmi")
            nc.vector.bn_stats(out=st6[r, :], in_=src[r, 0:ncols])
            nc.vector.bn_aggr(out=mv[r, :], in_=st6[r, :])
            # inv = 1/sqrt(var+eps)
            nc.vector.tensor_scalar_add(inv[r, :], mv[r, 1:2], 1e-5)
            nc.scalar.activation(inv[r, :], inv[r, :], AF.Sqrt)
            nc.vector.reciprocal(inv[r, :], inv[r, :])
            # nmi = -mean*inv
            nc.vector.tensor_tensor(nmi[r, :], mv[r, 0:1], inv[r, :],
                                    op=ALU.mult)
            nc.vector.tensor_scalar_mul(nmi[r, :], nmi[r, :], -1.0)
            return inv, nmi

        def _ln_apply(dst, src, rows, inv, nmi):
            nc.scalar.activation(dst[rows, :], src[rows, :], AF.Identity,
                                 scale=inv[rows, :], bias=nmi[rows, :])

        def transpose4(src, dst_t, dst_col, kstride, ncols=P, ident_n=P):
            """token-major [128,512] -> 4 feature-major chunks into dst_t,
            chunk kc at cols [kc*kstride + dst_col ...)."""
            pt = psT.tile([P, 512], src.dtype, tag="pst", name="pst")
            idt = ident8 if src.dtype == FP8 else ident
            for kc in range(4):
                nc.tensor.transpose(pt[:, kc * P:kc * P + ncols],
                                    src[:, kc * P:(kc + 1) * P],
                                    idt[0:ident_n, 0:ident_n])
            src3 = pt.rearrange("p (k c) -> p k c", c=P)[:, :, 0:ncols]
            dst3 = dst_t.rearrange("p (k c) -> p k c",
                                   c=kstride)[:, :, dst_col:dst_col + ncols]
            nc.scalar.activation(dst3, src3, AF.Copy)

        # ---- weights ----
        wc1t = [wp.tile([P, 2 * FF], FP8, tag=f"wc1{i}", name=f"wc1{i}")
                for i in range(2)]
        wc2t = [wp.tile([P, 2 * DIM], FP8, tag=f"wc2{i}", name=f"wc2{i}")
                for i in range(8)]
        wq = [wp.tile([P, DIM], BF16, tag=f"wq{i}", name=f"wq{i}")
              for i in range(4)]
        wk = [wp.tile([P, DIM], BF16, tag=f"wk{i}", name=f"wk{i}")
              for i in range(4)]
        wv = [wp.tile([P, DIM], BF16, tag=f"wv{i}", name=f"wv{i}")
              for i in range(4)]
        wo = [wp.tile([P, DIM], BF16, tag=f"wo{i}", name=f"wo{i}")
              for i in range(4)]
        ident = wp.tile([P, P], BF16, tag="ident", name="ident")
        nc.sync.dma_start(ident[:, :], id_d[:, :])
        sc16 = wp.tile([P, 1], F32, tag="sc16", name="sc16")
        nc.vector.memset(sc16[:, :], 1.0 / WSCALE)
        ident8 = wp.tile([P, P], FP8, tag="ident8", name="ident8")
        nc.sync.dma_start(ident8[:, :], id8_d[:, :])
        # x-FFN weights live in the attention-scratch rings (those rings
        # first rotate into real use only after the x-FFN has consumed
        # these tiles).
        w1g = [scrp.tile([P, J], BF16, tag="w", name=f"w1g{i}", bufs=2)
               for i in range(2)] + \
              [scrp.tile([P, J], BF16, tag="em", name=f"w1g{i+2}", bufs=2)
               for i in range(2)]
        w2p = [scrp.tile([P, J], BF16, tag="ge", name=f"w2p{i}", bufs=2)
               for i in range(2)] + \
              [scrp.tile([P, J], BF16, tag="attnf", name=f"w2p{i+2}",
                         bufs=2) for i in range(2)]
        w2h = [w2p[m // 4][:, (m % 4) * DIM:(m % 4 + 1) * DIM]
               for m in range(16)]
        # ctx-FFN weights first (group 0 starts on them), then the rest
        for i in range(2):
            nc.sync.dma_start(wc1t[i][:, :], wc1_d[i * P:(i + 1) * P, :])
        for i in range(8):
            nc.sync.dma_start(wc2t[i][:, :], wc2_d[i * P:(i + 1) * P, :])
        for i in range(4):
            nc.sync.dma_start(wk[i][:, :], wk_d[i * P:(i + 1) * P, :])
            nc.sync.dma_start(wv[i][:, :], wv_d[i * P:(i + 1) * P, :])
        for i in range(4):
            nc.sync.dma_start(w1g[i][:, 0:FF], w1g_d[i * P:(i + 1) * P, :])
            nc.sync.dma_start(wq[i][:, :], wq_d[i * P:(i + 1) * P, :])
        for m in range(16):
            nc.sync.dma_start(w2h[m], w2h_d[m * P:(m + 1) * P, :])
        for i in range(4):
            nc.sync.dma_start(wo[i][:, :], wo_d[i * P:(i + 1) * P, :])

        # persistent activations
        x2 = [pers.tile([P, DIM], F32, tag=f"x2_{t}", name=f"x2_{t}")
              for t in range(2)]
        qT = [pers.tile([P, BLOC * N], BF16, tag=f"qT{h}", name=f"qT{h}")
              for h in range(H)]
        kT = [pers.tile([P, J], BF16, tag=f"kT{h}", name=f"kT{h}")
              for h in range(H)]
        vsb = [pers.tile([P, 16 * DIM], BF16, tag=f"vsb{t}", name=f"vsb{t}")
               for t in range(2)]
        swT = [pers.tile([P, 2 * 512], FP8, tag=f"swT{m}", name=f"swT{m}")
               for m in range(8)]

        def emit_xffn_q():
            xt = [ctp.tile([P, DIM], F32, tag="ct", name="ct")
                  for _ in range(2)]
            for t in range(2):
                nc.sync.dma_start(xt[t][:, :], xin[t * P:(t + 1) * P, :])
            lnTx = ttp.tile([P, 4 * 256], BF16, tag="lnTx", name="lnTx",
                            bufs=1)
            for t in range(2):
                lno = lnp.tile([P, DIM], BF16, tag="lnout", name="lnout")
                inv, nmi = _ln_stats(xt[t], slice(0, P), DIM)
                _ln_apply(lno, xt[t], slice(0, P), inv, nmi)
                transpose4(lno, lnTx, t * P, 256)
            swx = [lnp.tile([P, 256], BF16, tag=f"swx{m}", name=f"swx{m}",
                            bufs=1) for m in range(16)]
            for m in range(16):
                hps = psH.tile([P, 512], F32, tag="psh", name="psh")
                for kc in range(4):
                    nc.tensor.matmul(
                        hps[:, 0:256],
                        w1g[kc][:, m * P:(m + 1) * P],
                        lnTx[:, kc * 256:(kc + 1) * 256],
                        start=(kc == 0), stop=(kc == 3))
                nc.scalar.activation(swx[m][:, :], hps[:, 0:256], AF.Silu)
            for t in range(2):
                fps = psH.tile([P, 512], F32, tag="psh", name="psh")
                for m in range(16):
                    nc.tensor.matmul(fps[:, :],
                                     swx[m][:, t * P:(t + 1) * P],
                                     w2h[m],
                                     start=(m == 0), stop=(m == 15))
                nc.vector.tensor_tensor(x2[t][:, :], fps[:, :],
                                        xt[t][:, :], op=ALU.add)
            aT = ttp.tile([P, 4 * 256], BF16, tag="lnTx", name="aT",
                          bufs=1)
            for t in range(2):
                a_bf = lnp.tile([P, DIM], BF16, tag="lnout", name="lnout")
                inv, nmi = _ln_stats(x2[t], slice(0, P), DIM)
                _ln_apply(a_bf, x2[t], slice(0, P), inv, nmi)
                transpose4(a_bf, aT, t * P, 256)
            for h in range(H):
                qps = psH.tile([P, 512], F32, tag="psh", name="psh")
                for kc in range(4):
                    nc.tensor.matmul(qps[:, 0:256],
                                     wq[kc][:, h * P:(h + 1) * P],
                                     aT[:, kc * 256:(kc + 1) * 256],
                                     start=(kc == 0), stop=(kc == 3))
                nc.scalar.activation(qT[h][:, :], qps[:, 0:256], AF.Copy)

        dots_t = {}   # (b, pair) -> tile

        def emit_group(b, g, hook=None):
            """ctx FFN + KV projection + dots chunk for batch b, group g
            (512 ctx tokens). fp8 DoubleRow for the FFN GEMMs. `hook` is
            called mid-group (after the Silu stage) to interleave
            attention work into the engine queues at fine granularity."""
            base = b * J + g * 512
            cts = [ctp.tile([P, DIM], F32, tag="ct", name="ct")
                   for _ in range(4)]
            for r in range(4):
                nc.sync.dma_start(
                    cts[r][:, :],
                    ctxin[base + r * P: base + (r + 1) * P, :])
            lnT = ttp.tile([P, 4 * 512], FP8, tag="lnTc", name="lnTc")
            for r in range(4):
                lno = lnp.tile([P, DIM], BF16, tag="lnout", name="lnout")
                inv, nmi = _ln_stats(cts[r], slice(0, P), DIM)
                _ln_apply(lno, cts[r], slice(0, P), inv, nmi)
                transpose4(lno, lnT, r * P, 512)
            for m in range(16):
                hps = psH.tile([P, 512], F32, tag="psh", name="psh")
                for k2 in range(2):
                    lhs = wc1t[k2].rearrange("p (two m) -> p two m",
                                             two=2)[:, :, m * P:(m + 1) * P]
                    rhs = lnT[:, k2 * 1024:(k2 + 1) * 1024].rearrange(
                        "p (two n) -> p two n", two=2)
                    nc.tensor.matmul(hps[:, :], lhs, rhs,
                                     start=(k2 == 0), stop=(k2 == 1),
                                     perf_mode=DR)
                nc.scalar.activation(swT[m // 2][:, (m % 2) * 512:
                                                 (m % 2 + 1) * 512],
                                     hps[:, :], AF.Silu,
                                     scale=sc16[:, :])
            if hook is not None:
                hook()
            c2s = [lnp.tile([P, DIM], BF16, tag="c2o", name="c2o")
                   for _ in range(4)]
            for t in range(4):
                fps = psH.tile([P, 512], F32, tag="psh", name="psh")
                for mp in range(8):
                    lhs = swT[mp].rearrange(
                        "p (two n) -> p two n",
                        two=2)[:, :, t * P:(t + 1) * P]
                    rhs = wc2t[mp].rearrange("p (two n) -> p two n", two=2)
                    nc.tensor.matmul(fps[:, :], lhs, rhs,
                                     start=(mp == 0), stop=(mp == 7),
                                     perf_mode=DR)
                tmp = lnp.tile([P, DIM], F32, tag="ftmp", name="ftmp")
                nc.scalar.activation(tmp[:, :], fps[:, :], AF.Copy,
                                     scale=sc16[:, :])
                nc.vector.tensor_tensor(c2s[t][:, :], tmp[:, :],
                                        cts[t][:, :], op=ALU.add)
            c2T = ttp.tile([P, 4 * 512], BF16, tag="c2T", name="c2T")
            for r in range(4):
                transpose4(c2s[r], c2T, r * P, 512)
            for h in range(H):
                kps = psH.tile([P, 512], F32, tag="psh", name="psh")
                for kc in range(4):
                    nc.tensor.matmul(kps[:, :],
                                     wk[kc][:, h * P:(h + 1) * P],
                                     c2T[:, kc * 512:(kc + 1) * 512],
                                     start=(kc == 0), stop=(kc == 3))
                nc.scalar.activation(kT[h][:, g * 512:(g + 1) * 512],
                                     kps[:, :], AF.Copy)
            vdst = vsb[b % 2]
            for t in range(4):
                vps = psH.tile([P, 512], F32, tag="psh", name="psh")
                for kc in range(4):
                    nc.tensor.matmul(
                        vps[:, :],
                        c2T[:, kc * 512 + t * P:kc * 512 + (t + 1) * P],
                        wv[kc][:, :],
                        start=(kc == 0), stop=(kc == 3))
                rt = g * 4 + t
                nc.vector.tensor_copy(vdst[:, rt * DIM:(rt + 1) * DIM],
                                      vps[:, :])
            if b > 0:
                emit_dots(b, g)

        def emit_dots(b, g):
            for pair in range(2):
                if g == 0:
                    if pair == 0:
                        dots_t[(b, 0)] = dotp.tile([P, J], BF16,
                                                   tag="dots", name="dots")
                    else:
                        dots_t[(b, 1)] = dotp.tile([P, J], BF16,
                                                   tag="dots", name="dots")
                dots = dots_t[(b, pair)]
                for hi in range(2):
                    h = 2 * pair + hi
                    dps = psD.tile([64, 512], F32, tag="psd", name="psd")
                    nc.tensor.matmul(dps[:, :],
                                     qT[h][:, b * N:(b + 1) * N],
                                     kT[h][:, g * 512:(g + 1) * 512],
                                     start=True, stop=True)
                    nc.scalar.activation(
                        dots[hi * 64:(hi + 1) * 64,
                             g * 512:(g + 1) * 512],
                        dps[:, :], AF.Copy)

        att_st = {}   # (b, pair) -> dict of tiles across slices

        def emit_topk_part(b, pair, part):
            """part 0: copy + rounds 0-3; part 1: rounds 4-7 + thr + ge."""
            dots = dots_t[(b, pair)]
            if part == 0:
                st = att_st[(b, pair)] = {}
                w = scrp.tile([P, J], BF16, tag="w", name="w")
                nc.vector.tensor_copy(w[:, :], dots[:, :])
                mx = smp.tile([P, 64], BF16, tag="mx", name="mx")
                st['w'] = w
                st['mx'] = mx
                rounds = range(0, 4)
            else:
                st = att_st[(b, pair)]
                w, mx = st['w'], st['mx']
                rounds = range(4, 8)
            for r8 in rounds:
                nc.vector.max(mx[:, r8 * 8:(r8 + 1) * 8], w[:, :])
                if r8 < 7:
                    nc.vector.match_replace(
                        w[:, :], mx[:, r8 * 8:(r8 + 1) * 8], w[:, :],
                        -3.0e38)
            if part == 0:
                # prefetch band mask while rounds run
                band = scrp.tile([P, J], BF16, tag="ge", name="band")
                nc.sync.dma_start(
                    band[:, :],
                    band_d[(b * 2 + pair) * P:(b * 2 + pair + 1) * P, :])
                st['band'] = band
                return
            thr = mx[:, 63:64]
            thrf = smp.tile([P, 1], F32, tag="thrf", name="thrf")
            nc.vector.tensor_scalar_mul(thrf[:, :], thr, 1.0)
            negthr = smp.tile([P, 1], F32, tag="negthr", name="negthr")
            nc.vector.tensor_scalar_mul(negthr[:, :], thr, -1.0)
            st['negthr'] = negthr
            ge = scrp.tile([P, J], BF16, tag="attnf", name="ge")
            nc.vector.tensor_scalar(ge[:, :], dots[:, :], thrf[:, :], None,
                                    op0=ALU.is_ge)
            nc.gpsimd.tensor_tensor(ge[:, :], ge[:, :], st['band'][:, :],
                                    op=ALU.mult)
            st['ge'] = ge

        def emit_soft(b, pair, aout):
            dots = dots_t[(b, pair)]
            st = att_st[(b, pair)]
            em = scrp.tile([P, J], BF16, tag="em", name="em")
            nc.scalar.activation(em[:, :], dots[:, :], AF.Exp,
                                 bias=st['negthr'][:, :])
            nc.vector.tensor_tensor(em[:, :], em[:, :], st['ge'][:, :],
                                    op=ALU.mult)
            zS = smp.tile([P, 1], F32, tag="z", name="z")
            nc.vector.reduce_sum(zS[:, :], em[:, :], axis=AX.X)
            degS = smp.tile([P, 1], F32, tag="deg", name="deg")
            izS = smp.tile([P, 1], F32, tag="iz", name="iz")
            uS = smp.tile([P, 1], F32, tag="u", name="u")
            nc.vector.tensor_scalar(degS[:, :], zS[:, :], 0.5, None,
                                    op0=ALU.is_le)
            nc.vector.tensor_tensor(izS[:, :], zS[:, :], degS[:, :],
                                    op=ALU.add)
            nc.vector.reciprocal(izS[:, :], izS[:, :])
            nc.vector.tensor_scalar_mul(uS[:, :], degS[:, :], 1.0 / J)
            attnf = scrp.tile([P, J], BF16, tag="w", name="attnf")
            nc.vector.tensor_scalar(attnf[:, :], em[:, :], izS[:, :],
                                    uS[:, :], op0=ALU.mult, op1=ALU.add)
            # transpose attnf -> atT
            atT = scrp.tile([P, J], BF16, tag="em", name="atT")
            for jcg in range(4):
                pt = psT.tile([P, 512], BF16, tag="pst", name="pst")
                for j4 in range(4):
                    jc = jcg * 4 + j4
                    nc.tensor.transpose(pt[:, j4 * P:(j4 + 1) * P],
                                        attnf[:, jc * P:(jc + 1) * P],
                                        ident[:, :])
                nc.scalar.activation(atT[:, jcg * 512:(jcg + 1) * 512],
                                     pt[:, :], AF.Copy)
            avp = [psAV.tile([64, 512], F32, tag="av", name="av")
                   for _ in range(2)]
            vsrc = vsb[b % 2]
            for jc in range(16):
                for hi in range(2):
                    h = 2 * pair + hi
                    nc.tensor.matmul(
                        avp[hi][:, 0:P],
                        atT[:, jc * P + hi * 64:jc * P + hi * 64 + 64],
                        vsrc[:, jc * DIM + h * P:jc * DIM + (h + 1) * P],
                        start=(jc == 0), stop=(jc == 15))
            for hi in range(2):
                h = 2 * pair + hi
                nc.scalar.activation(aout[:, h * P:(h + 1) * P],
                                     avp[hi][:, 0:P], AF.Copy)

        def emit_out(b, aout):
            aoT = aop.tile([P, 256], BF16, tag="aoT", name="aoT")
            pt = psT.tile([P, 512], BF16, tag="pst", name="pst")
            for kc in range(4):
                nc.tensor.transpose(pt[:, kc * P:kc * P + 64],
                                    aout[:, kc * P:(kc + 1) * P],
                                    ident[0:64, 0:64])
            src3 = pt.rearrange("p (k c) -> p k c", c=P)[:, :, 0:64]
            dst3 = aoT.rearrange("p (k c) -> p k c", c=64)
            nc.scalar.activation(dst3, src3, AF.Copy)
            ops = psAV.tile([64, 512], F32, tag="av", name="av")
            for kc in range(4):
                nc.tensor.matmul(ops[:, :], aoT[:, kc * 64:(kc + 1) * 64],
                                 wo[kc][:, :],
                                 start=(kc == 0), stop=(kc == 3))
            xf = aop.tile([64, DIM], F32, tag="xf", name="xf")
            x2t = x2[b // 2]
            nc.vector.tensor_tensor(
                xf[:, :], ops[:, :],
                x2t[(b % 2) * 64:(b % 2) * 64 + 64, :], op=ALU.add)
            outn = aop.tile([64, DIM], F32, tag="outn", name="outn")
            inv, nmi = _ln_stats(xf, slice(0, 64), DIM)
            _ln_apply(outn, xf, slice(0, 64), inv, nmi)
            nc.sync.dma_start(outd[b * N:(b + 1) * N, :], outn[0:64, :])

        def att_slice(b, s, aout_t):
            """8 half-group-granularity slices per batch."""
            if s == 0:
                emit_topk_part(b, 0, 0)
            elif s == 1:
                emit_topk_part(b, 0, 1)
            elif s == 2:
                emit_soft(b, 0, aout_t[b])
            elif s == 3:
                emit_topk_part(b, 1, 0)
            elif s == 4:
                emit_topk_part(b, 1, 1)
            elif s == 5:
                emit_soft(b, 1, aout_t[b])
            elif s == 6:
                emit_out(b, aout_t[b])

        aout_t = {}
        for b in range(BLOC):
            aout_t[b] = aop.tile([64, 512], BF16, tag="aout", name="aout")
            for g in range(4):
                if b > 0:
                    emit_group(b, g, hook=(
                        lambda s=2 * g: att_slice(b - 1, s, aout_t)))
                    att_slice(b - 1, 2 * g + 1, aout_t)
                else:
                    emit_group(b, g)
            if b == 0:
                emit_xffn_q()
                for g in range(4):
                    emit_dots(0, g)
        for s in range(8):
            att_slice(BLOC - 1, s, aout_t)
    nc.compile()
    return nc


def _fold_weights(inputs):
    f32 = np.float32
    g1 = np.asarray(inputs['ln1_g'], f32)[:, None]
    gkv = np.asarray(inputs['lnkv_g'], f32)[:, None]
    ga = np.asarray(inputs['lna_g'], f32)[:, None]
    w1g = (g1 * np.asarray(inputs['ff1_w1'], f32)).astype(BF)
    w2h = (0.5 * np.asarray(inputs['ff1_w2'], f32)).astype(BF)
    wc1 = (gkv * np.asarray(inputs['ffkv_w1'], f32) * WSCALE)
    wc1 = wc1.reshape(2, 2, P, FF).transpose(0, 2, 1, 3).reshape(
        2 * P, 2 * FF).astype(F8)
    wc2 = (0.5 * np.asarray(inputs['ffkv_w2'], f32) * WSCALE)
    wc2 = wc2.reshape(8, 2, P, DIM).transpose(0, 2, 1, 3).reshape(
        8 * P, 2 * DIM).astype(F8)
    wq = (ga * np.asarray(inputs['wq'], f32) * (DH ** -0.5)).astype(BF)
    wkv = np.asarray(inputs['wkv'], f32)
    wk = np.ascontiguousarray(wkv[:, :DIM]).astype(BF)
    wv = np.ascontiguousarray(wkv[:, DIM:]).astype(BF)
    wo = np.asarray(inputs['wo'], f32).astype(BF)
    return w1g, w2h, wc1, wc2, wq, wk, wv, wo


def _band_table(core):
    """Multiplicative 0/1 local-band masks for this core's 4 batches x 2
    head-pairs: [BLOC*2*128, J] bf16, rows (b*2+pair)*128 + hi*64 + i."""
    m = np.ones((BLOC * 2 * P, J), np.float32)
    blk = np.arange(J) // N
    for b in range(BLOC):
        gbat = core * BLOC + b
        for pair in range(2):
            for hi in range(2):
                h = 2 * pair + hi
                L = PATTERN[h]
                if L is None:
                    continue
                bad = np.abs(blk - gbat) > L
                r0 = (b * 2 + pair) * P + hi * 64
                m[r0:r0 + 64, bad] = 0.0
    return m.astype(BF)


def _in_maps(inputs):
    x = np.asarray(inputs['x'], np.float32)
    ctxf = np.asarray(inputs['context'], np.float32)
    w1g, w2h, wc1, wc2, wq, wk, wv, wo = _fold_weights(inputs)
    ident = np.eye(P, dtype=BF)
    ident8 = np.eye(P, dtype=F8)
    in_maps = []
    for c in range(NCORES):
        bs = slice(c * BLOC, (c + 1) * BLOC)
        in_maps.append({
            'xin': np.ascontiguousarray(x[bs].reshape(BLOC * N, DIM)),
            'ctxin': np.ascontiguousarray(ctxf[bs].reshape(BLOC * J, DIM)),
            'w1g': w1g, 'w2h': w2h, 'wc1': wc1, 'wc2': wc2,
            'wq': wq, 'wk': wk, 'wv': wv, 'wo': wo,
            'ident': ident, 'ident8': ident8,
            'bands': _band_table(c),
        })
    return in_maps


def kernel(**inputs):
    from concourse.bass_utils import run_bass_kernel_spmd

    if 'nc' not in _CACHE:
        _CACHE['nc'] = build_bass()
    nc = _CACHE['nc']
    res = run_bass_kernel_spmd(nc, _in_maps(inputs), list(range(NCORES)))
    outs = [np.asarray(res.results[c]['out']).reshape(BLOC, N, DIM)
            for c in range(NCORES)]
    on = np.concatenate(outs, axis=0)
    g = np.asarray(inputs['lnf_g'], np.float32)
    bta = np.asarray(inputs['lnf_b'], np.float32)
    return (g * on + bta).astype(np.float32)
